# revision 23
# baseline (speedup 1.0000x reference)
"""Trainium2 Bass kernel for causal cross-attention (B=2,S=2048,D=1024,H=16).

Original staged baseline (164327 ns) — kept as a known-passing fallback.
"""

import sys

sys.path.insert(0, "/opt/trn_rl_repo")

import heapq
from contextlib import ExitStack

import ml_dtypes
import numpy as np

import concourse.bass as bass  # noqa: F401  (engine types via nc)
import concourse.tile as tile
from concourse import bacc
from concourse import mybir
from concourse.bass_utils import run_bass_kernel_spmd

B, S, D, H = 2, 2048, 1024, 16
HD = 64                      # head dim
SCALE = HD ** -0.5
HG = 4                       # heads per core
DL = HG * HD                 # 256 local projection dims per core
P = 128
NJ = S // 512                # 4 q chunks
NI = S // P                  # 16 k tiles / s tiles
KD = D // P                  # 8 din tiles
L_AV = 4                     # AV lag behind scores, in sc-steps

F32 = mybir.dt.float32
BF16 = mybir.dt.bfloat16
EXP = mybir.ActivationFunctionType.Exp
BF = ml_dtypes.bfloat16

_PROG = None


def _build_program():
    nc = bacc.Bacc()
    p_xT = nc.declare_dram_parameter("xT", [D, S], BF16, isOutput=False)
    p_yT = nc.declare_dram_parameter("yT", [D, S], BF16, isOutput=False)
    p_wq = nc.declare_dram_parameter("wq", [D, DL], BF16, isOutput=False)
    p_wk = nc.declare_dram_parameter("wk", [D, DL], BF16, isOutput=False)
    p_wv = nc.declare_dram_parameter("wv", [D, DL], BF16, isOutput=False)
    p_wo = nc.declare_dram_parameter("wo", [2, P, D], BF16, isOutput=False)
    p_bq = nc.declare_dram_parameter("bq", [DL], F32, isOutput=False)
    p_bk = nc.declare_dram_parameter("bk", [DL], F32, isOutput=False)
    p_masks = nc.declare_dram_parameter("masks", [HG, P, 512], BF16, isOutput=False)
    p_ones = nc.declare_dram_parameter("ones_c", [P, 8], BF16, isOutput=False)
    p_out = nc.declare_dram_parameter("out", [S, D], BF16, isOutput=True)

    with tile.TileContext(nc) as tc, ExitStack() as ctx:
        singles = ctx.enter_context(tc.tile_pool(name="singles", bufs=1))
        scp = ctx.enter_context(tc.tile_pool(name="scp", bufs=2, space="PSUM"))
        avp = ctx.enter_context(tc.tile_pool(name="avp", bufs=1, space="PSUM"))
        dnp = ctx.enter_context(tc.tile_pool(name="dnp", bufs=1, space="PSUM"))
        mip = ctx.enter_context(tc.tile_pool(name="mip", bufs=2, space="PSUM"))
        epool = ctx.enter_context(tc.tile_pool(name="e", bufs=6))
        npool = ctx.enter_context(tc.tile_pool(name="natt", bufs=6))
        rpool = ctx.enter_context(tc.tile_pool(name="rec", bufs=3))
        opool = ctx.enter_context(tc.tile_pool(name="outsb", bufs=2))
        ttpool = ctx.enter_context(tc.tile_pool(name="oTT", bufs=4))

        def emit_body():
            xall = singles.tile([P, KD, S], BF16, tag="xall")
            yall = singles.tile([P, KD, S], BF16, tag="yall")
            wq_sb = singles.tile([P, KD, DL], BF16, tag="wq")
            wk_sb = singles.tile([P, KD, DL], BF16, tag="wk")
            wv_sb = singles.tile([P, KD, DL], BF16, tag="wv")
            wo_sb = singles.tile([P, 2, D], BF16, tag="wo")
            bq_sb = singles.tile([P, 2], F32, tag="bq")
            bk_sb = singles.tile([P, 2], F32, tag="bk")
            masks = singles.tile([P, HG, 512], BF16, tag="masks")
            ones = singles.tile([P, 8], BF16, tag="ones")
            qT = singles.tile([P, 2, S], BF16, tag="qT")
            kT = singles.tile([P, 2, S], BF16, tag="kT")
            vsb = [singles.tile([P, DL], BF16, tag=f"v{i}", name=f"v_{i}")
                   for i in range(NI)]
            dn = dnp.tile([P, 64], F32, tag="dn")

            wq_r = p_wq.rearrange("(o p) m -> p o m", p=P)
            wk_r = p_wk.rearrange("(o p) m -> p o m", p=P)
            wv_r = p_wv.rearrange("(o p) m -> p o m", p=P)
            nc.scalar.dma_start(wq_sb[:, 0:KD // 2], wq_r[:, 0:KD // 2])
            nc.scalar.dma_start(bq_sb, p_bq.rearrange("(o p) -> p o", p=P))
            nc.scalar.dma_start(wk_sb[:, 0:KD // 2], wk_r[:, 0:KD // 2])
            nc.scalar.dma_start(bk_sb, p_bk.rearrange("(o p) -> p o", p=P))
            nc.scalar.dma_start(wq_sb[:, KD // 2:KD], wq_r[:, KD // 2:KD])
            nc.scalar.dma_start(wk_sb[:, KD // 2:KD], wk_r[:, KD // 2:KD])
            nc.scalar.dma_start(wv_sb, wv_r)
            nc.scalar.dma_start(ones, p_ones[:, :])
            nc.scalar.dma_start(masks, p_masks.rearrange("m p q -> p m q"))
            nc.scalar.dma_start(wo_sb, p_wo.rearrange("o p m -> p o m"))
            x_r = p_xT.rearrange("(a p) q -> p a q", p=P)
            y_r = p_yT.rearrange("(a p) q -> p a q", p=P)
            for c0, c1 in ((0, 512), (512, 1024), (1024, 2048)):
                nc.sync.dma_start(xall[:, :, c0:c1], x_r[:, :, c0:c1])
                nc.sync.dma_start(yall[:, :, c0:c1], y_r[:, :, c0:c1])

            pending = []
            seqc = [0]
            gstep = [0]

            def push(min_step, fn):
                heapq.heappush(pending, (min_step, seqc[0], fn))
                seqc[0] += 1

            def drain():
                while pending and pending[0][0] <= gstep[0]:
                    heapq.heappop(pending)[2]()

            def proj_items(j, w_sb, b_sb, res, dst, nm):
                state = {}

                def lo():
                    state["pair"] = scp.tile([P, 1024], F32, tag="sp",
                                             name=f"pp_{nm}_{j}")
                    pair = state["pair"]
                    for i in range(KD // 2):
                        for d in range(2):
                            nc.tensor.matmul(
                                pair[:, 512 * d:512 * d + 512],
                                lhsT=w_sb[:, i, P * d:P * d + P],
                                rhs=res[:, i, 512 * j:512 * j + 512],
                                start=(i == 0), stop=False)

                def hi():
                    pair = state["pair"]
                    for i in range(KD // 2, KD):
                        for d in range(2):
                            nc.tensor.matmul(
                                pair[:, 512 * d:512 * d + 512],
                                lhsT=w_sb[:, i, P * d:P * d + P],
                                rhs=res[:, i, 512 * j:512 * j + 512],
                                start=False, stop=(i == KD - 1))
                    for d in range(2):
                        nc.vector.tensor_scalar_add(
                            dst[:, d, 512 * j:512 * j + 512],
                            pair[:, 512 * d:512 * d + 512],
                            b_sb[:, d:d + 1])

                return lo, hi

            def emit_proj_full(j, w_sb, b_sb, res, dst, nm):
                a, b = proj_items(j, w_sb, b_sb, res, dst, nm)
                a()
                b()

            def vblock_items(jb):
                state = {}
                items = []

                def mk(i):
                    def it():
                        if i == 0:
                            state["b"] = [
                                mip.tile([P, 512], F32, tag="mi",
                                         name=f"vb_{jb}_{h}")
                                for h in range(2)]
                        for sv in range(4):
                            nc.tensor.matmul(
                                state["b"][sv // 2][:, 256 * (sv % 2):
                                                    256 * (sv % 2) + 256],
                                lhsT=yall[:, i, P * (4 * jb + sv):
                                          P * (4 * jb + sv) + P],
                                rhs=wv_sb[:, i, :],
                                start=(i == 0 and sv % 2 == 0),
                                stop=(i == KD - 1),
                                skip_group_check=(sv % 2 == 1))
                    return it

                for i in range(KD):
                    items.append(mk(i))

                def drain_it(sv):
                    def it():
                        nc.vector.tensor_copy(
                            out=vsb[4 * jb + sv],
                            in_=state["b"][sv // 2][:, 256 * (sv % 2):
                                                    256 * (sv % 2) + 256])
                    return it
                for sv in range(4):
                    items.append(drain_it(sv))
                return items

            def op_items(j, tt, oTT_pair):
                t = 4 * j + tt
                state = {}

                def mk(n):
                    def it():
                        if n == 0:
                            state["o"] = opool.tile([P, D], BF16, tag="osb",
                                                    name=f"osb_{t}")
                        bank = mip.tile([P, 512], F32, tag="mi",
                                        name=f"op_{t}_{n}")
                        for hp in range(2):
                            nc.tensor.matmul(
                                bank,
                                lhsT=oTT_pair[hp][:, P * tt:P * tt + P],
                                rhs=wo_sb[:, hp, 512 * n:512 * n + 512],
                                start=(hp == 0), stop=(hp == 1))
                        if n == 0:
                            nc.vector.tensor_copy(
                                out=state["o"][:, 0:512], in_=bank)
                        else:
                            nc.scalar.copy(
                                state["o"][:, 512:1024], bank)
                        if n == 1:
                            nc.gpsimd.dma_start(p_out[P * t:P * t + P, :],
                                                state["o"])
                    return it

                return [mk(0), mk(1)]

            def emit_unit(j, hp):
                nk = 4 * j + 4
                avb = avp.tile([P, 512], F32, tag="avb", name=f"avb_{j}_{hp}")
                rec = rpool.tile([P, 8], F32, tag="rec", name=f"rec_{j}_{hp}")
                c0 = 16 * j + 8 * hp
                natts = {}
                oTT_t = ttpool.tile([P, 512], BF16, tag="ott",
                                    name=f"ott_{j}_{hp}")

                def av_step(i, sl):
                    m = i - 4 * j

                    def it():
                        first_dn = (j == 0 and hp == 0 and i == 0)
                        for par in range(2):
                            for tt in range(4):
                                if 0 <= m and tt < m:
                                    continue
                                g = 4 * par + tt
                                nc.tensor.matmul(
                                    avb[:, HD * g:HD * g + HD],
                                    lhsT=sl(par, tt),
                                    rhs=vsb[i][:, HD * (2 * hp + par):
                                               HD * (2 * hp + par) + HD],
                                    start=(i == 0 and g == 0),
                                    stop=(i == 4 * j + tt),
                                    skip_group_check=(g != 0))
                                nc.tensor.matmul(
                                    dn[:, c0 + g:c0 + g + 1],
                                    lhsT=sl(par, tt),
                                    rhs=ones[:, 0:1],
                                    start=(first_dn and g == 0),
                                    stop=(i == 4 * j + tt),
                                    skip_group_check=True)
                    return it

                def recip_item(tt):
                    def it():
                        nc.vector.reciprocal(
                            rec.rearrange("p (a b) -> p a b", a=2)[:, :, tt],
                            dn.rearrange("p (u a b) -> p u a b", a=2, b=4)[
                                :, c0 // 8, :, tt])
                    return it

                def norm_item(tt):
                    def it():
                        nt = npool.tile([P, P], BF16, tag="nt",
                                        name=f"nt_{j}_{hp}_{tt}")
                        natts[tt] = nt
                        for par in range(2):
                            g = 4 * par + tt
                            nc.vector.tensor_scalar_mul(
                                nt[:, HD * par:HD * par + HD],
                                avb[:, HD * g:HD * g + HD],
                                rec[:, g:g + 1])
                        nc.sync.dma_start_transpose(
                            oTT_t[:, P * tt:P * tt + P], nt)
                    return it

                pair_sp = {}

                def interior_slicer(e_pp, ih):
                    return lambda par, tt: e_pp[par][:, 512 * ih + P * tt:
                                                     512 * ih + P * tt + P]

                def diag_slicer(e_sb):
                    return lambda par, tt: e_sb[:, 512 * par + P * tt:
                                                512 * par + P * tt + P]

                for i in range(nk):
                    m = i - 4 * j
                    if m < 0:
                        ih = i % 2
                        if ih == 0:
                            pair_sp = {
                                par: scp.tile([P, 1024], F32, tag="sp",
                                              name=f"sp_{j}_{hp}_{i}_{par}")
                                for par in range(2)}
                        for par in range(2):
                            base = HD * par
                            nc.tensor.matmul(
                                pair_sp[par][:, 512 * ih:512 * ih + 512],
                                lhsT=kT[base:base + HD, hp, P * i:P * i + P],
                                rhs=qT[base:base + HD, hp,
                                       512 * j:512 * j + 512],
                                start=True, stop=True)
                        if ih == 1:
                            e_pp = {}
                            for par in range(2):
                                e_pp[par] = epool.tile(
                                    [P, 1024], BF16, tag="e",
                                    name=f"e_pp_{j}_{hp}_{i}_{par}")
                                nc.scalar.activation(
                                    e_pp[par], pair_sp[par], EXP, scale=SCALE)
                            push(gstep[0] + L_AV - 1,
                                 av_step(i - 1, interior_slicer(e_pp, 0)))
                            push(gstep[0] + L_AV,
                                 av_step(i, interior_slicer(e_pp, 1)))
                    else:
                        lo = m * P
                        sp = scp.tile([P, 1024], F32, tag="sp",
                                      name=f"sp_{j}_{hp}_{i}")
                        for par in range(2):
                            base = HD * par
                            nc.tensor.matmul(
                                sp[:, 512 * par + lo:512 * par + 512],
                                lhsT=kT[base:base + HD, hp, P * i:P * i + P],
                                rhs=qT[base:base + HD, hp,
                                       512 * j + lo:512 * j + 512],
                                start=True, stop=True)
                        e_sb = epool.tile([P, 1024], BF16, tag="e")
                        ev = e_sb.rearrange("p (g q) -> p g q", g=2)[:, :, lo:]
                        nc.scalar.activation(
                            ev, sp.rearrange("p (g q) -> p g q", g=2)[
                                :, :, lo:],
                            EXP, scale=SCALE)
                        nc.vector.tensor_mul(
                            ev, ev,
                            masks[:, m:m + 1, lo:].to_broadcast(
                                (P, 2, 512 - lo)))
                        push(gstep[0] + L_AV, av_step(i, diag_slicer(e_sb)))
                    gstep[0] += 1
                    drain()
                for tt in range(4):
                    push(gstep[0] - (3 - tt) + L_AV, recip_item(tt))
                    push(gstep[0] - (3 - tt) + L_AV, norm_item(tt))
                return oTT_t

            emit_proj_full(0, wq_sb, bq_sb, xall, qT, "q")
            emit_proj_full(0, wk_sb, bk_sb, yall, kT, "k")
            for k, it in enumerate(vblock_items(0)):
                push(0, it)

            oTTs = {}
            for j in range(NJ):
                start_step = gstep[0]
                if j + 1 < NJ:
                    qlo, qhi = proj_items(j + 1, wq_sb, bq_sb, xall, qT, "q")
                    klo, khi = proj_items(j + 1, wk_sb, bk_sb, yall, kT, "k")
                    push(start_step + 1, qlo)
                    push(start_step + 3, qhi)
                    push(start_step + 5, klo)
                    push(start_step + 7, khi)
                    vits = vblock_items(j + 1)
                    for k, it in enumerate(vits):
                        push(start_step + 2 + k, it)
                for hp in range(2):
                    oTTs[(j, hp)] = emit_unit(j, hp)
                    gstep[0] += 1
                    drain()
                ops = []
                for tt in range(4):
                    ops.extend(op_items(
                        j, tt, [oTTs[(j, 0)], oTTs[(j, 1)]]))
                for k, it in enumerate(ops):
                    push(gstep[0] + L_AV + 4 + k, it)

            while pending:
                heapq.heappop(pending)[2]()

        emit_body()
    nc.compile()
    return nc


def _get_program():
    global _PROG
    if _PROG is None:
        _PROG = _build_program()
    return _PROG


def _build_masks():
    q = np.arange(512)[None, :]
    k = np.arange(P)[:, None]
    return np.stack(
        [(q >= k + P * m) for m in range(HG)], axis=0).astype(BF)


def _prep_in_maps(inputs):
    x = np.asarray(inputs["x"], np.float32)
    y = np.asarray(inputs["y"], np.float32)
    Wq = np.asarray(inputs["Wq"], np.float32)
    Wk = np.asarray(inputs["Wk"], np.float32)
    Wv = np.asarray(inputs["Wv"], np.float32)
    Wo = np.asarray(inputs["Wo"], np.float32)
    bq = np.asarray(inputs["bq"], np.float32)
    bk = np.asarray(inputs["bk"], np.float32)

    masks = _build_masks()
    ones_c = np.ones((P, 8), BF)
    xTs = [np.ascontiguousarray(x[b].T).astype(BF) for b in range(B)]
    yTs = [np.ascontiguousarray(y[b].T).astype(BF) for b in range(B)]

    in_maps = []
    for c in range(8):
        b, hg = divmod(c, HG)
        sl = slice(DL * hg, DL * hg + DL)
        wo_loc = Wo[sl, :].reshape(HG, HD, D)
        wo2 = np.stack([wo_loc[2 * hp:2 * hp + 2].reshape(P, D)
                        for hp in range(2)]).astype(BF)
        in_maps.append({
            "xT": xTs[b],
            "yT": yTs[b],
            "wq": np.ascontiguousarray(Wq[:, sl]).astype(BF),
            "wk": np.ascontiguousarray(Wk[:, sl]).astype(BF),
            "wv": np.ascontiguousarray(Wv[:, sl]).astype(BF),
            "wo": wo2,
            "bq": np.ascontiguousarray(bq[sl]),
            "bk": np.ascontiguousarray(bk[sl]),
            "masks": masks,
            "ones_c": ones_c,
        })
    return in_maps


def run(inputs, trace=False):
    bv = np.asarray(inputs["bv"], np.float32)
    bo = np.asarray(inputs["bo"], np.float32)
    Wo = np.asarray(inputs["Wo"], np.float32)

    nc = _get_program()
    in_maps = _prep_in_maps(inputs)

    res = run_bass_kernel_spmd(nc, in_maps, list(range(8)), trace=trace)
    extra = bv @ Wo + bo
    out = np.empty((B, S, D), np.float32)
    for b in range(B):
        acc = res.results[HG * b]["out"].astype(np.float32)
        for hg in range(1, HG):
            acc = acc + res.results[HG * b + hg]["out"].astype(np.float32)
        out[b] = acc + extra
    return out, res


def kernel(**inputs):
    out, _ = run(inputs, trace=False)
    return out
